# revision 1
# baseline (speedup 1.0000x reference)
# Mistral sliding-window attention (B=1, S=2048, H=4096, 32 q heads / 8 kv
# heads, window 4096 -> plain causal at this S) on 8 Trainium2 NeuronCores.
#
# Sharding: tensor-parallel over heads. Core c owns q heads 4c..4c+3 and kv
# head c. hidden_states is replicated (transposed on host to [H, S] so the
# contraction dim is the partition dim). Each core computes its attention
# output slice attn.T [512, S]; per-head AllGathers assemble the full
# [4096, S] while later heads still compute, and each core accumulates a
# 512-column slice of o_proj head-by-head; the host concatenates the 8
# column slices into the full output.
#
# All big matmuls run as float32r (fp32 storage, full-rate PE) with the
# moving dim = 512. Scores are computed transposed (S.T[kv, q]) so that the
# P@V contraction needs no transposes of the probability tiles; softmax
# denominators come from an all-ones stationary matmul accumulated alongside
# P@V, and the causal mask is a host-precomputed staircase slice multiplied
# in after exp. Attention runs two-pass per (head, q-chunk) — all score
# matmuls + exps first, then the PV/sum matmuls — so the PE never stalls on
# the ACT engine mid-chain.

from contextlib import ExitStack

import numpy as np

import concourse.bacc as bacc
import concourse.bass as bass
import concourse.mybir as mybir
import concourse.tile as tile
from concourse.bass_utils import run_bass_kernel_spmd
from concourse.masks import make_identity

HIDDEN = 4096
NH = 32
NKV = 8
HD = 128
THETA = 10000.0
S = 2048
NCORES = 8

QH = NH // NCORES          # 4 q heads per core
DQ = QH * HD               # 512 (per-core q/attn width)
DOUT = DQ + 2 * HD         # 768 = q heads + k + v projection width
MT = DOUT // 128           # 6 projection m-tiles (0..3 q, 4 k, 5 v)
KT = HIDDEN // 128         # 32 contraction tiles
KG = 4                     # x-load group: k-tiles per DMA
TCH = 512                  # token chunk (matmul moving dim)
NTCH = S // TCH            # 4
KVT = S // 128             # 16 kv tiles
SCALE = 1.0 / float(np.sqrt(HD))

F32 = mybir.dt.float32
F32R = mybir.dt.float32r
EXP = mybir.ActivationFunctionType.Exp


def _rope(nc, pool, src, dst, cs, sn):
    """dst = src*cos + rotate_half(src)*sin, in [d, tok] layout.

    src/dst are [128, n]; cs/sn are [64, n] (the two 64-row halves share
    frequencies). rotate_half: rows 0:64 get -src[64:128], rows 64:128 get
    src[0:64].
    """
    top, bot = src[0:64, :], src[64:128, :]
    ta = pool.tile([64, TCH], F32, name="rope_a")
    tb = pool.tile([64, TCH], F32, name="rope_b")
    nc.vector.tensor_mul(ta, top, cs)
    nc.vector.tensor_mul(tb, bot, sn)
    nc.vector.tensor_sub(dst[0:64, :], ta, tb)
    nc.vector.tensor_mul(ta, bot, cs)
    nc.vector.tensor_mul(tb, top, sn)
    nc.vector.tensor_add(dst[64:128, :], ta, tb)


def build_kernel_body(ctx: ExitStack, tc: tile.TileContext, outs, ins):
    nc = tc.nc
    xT, wqkv, ow, cos_t, sin_t, stair = (
        ins["xT"], ins["wqkv"], ins["ow"], ins["cos_t"], ins["sin_t"], ins["stair"],
    )
    out = outs["out"]

    # per-head bounce + gather buffers so each head's AllGather can fire as
    # soon as that head's attention is done (overlaps comm with compute)
    attn_loc = [nc.dram_tensor(f"attn_loc{h}", [HD, S], F32).ap()
                for h in range(QH)]
    attn_gat = [nc.dram_tensor(f"attn_gat{h}", [NCORES * HD, S], F32,
                               addr_space="Shared").ap()
                for h in range(QH)]

    singles = ctx.enter_context(tc.tile_pool(name="singles", bufs=1))
    stair_sb = singles.tile([128, 896], F32)
    nc.sync.dma_start(out=stair_sb, in_=stair)
    ones_sb = singles.tile([128, 128], F32R)

    # persistent projection outputs, [d, tok] layout
    qT = singles.tile([128, QH, S], F32R)    # q head h -> qT[:, h, :]
    kT = singles.tile([128, S], F32R)
    V = singles.tile([128, KVT, HD], F32R)   # V[:, j, :] = [tok 128, d 128]

    # ---- phase 1: QKV projection + RoPE --------------------------------
    with (
        tc.tile_pool(name="wq", bufs=1) as wp,
        tc.tile_pool(name="xt", bufs=3) as xp,
        tc.tile_pool(name="rope", bufs=2) as rp,
        tc.tile_pool(name="p1ps", bufs=1, space="PSUM") as pp1,
    ):
        cos_sb = wp.tile([64, S], F32)
        sin_sb = wp.tile([64, S], F32)
        vT = wp.tile([128, S], F32)
        ident_sb = wp.tile([128, 128], F32)
        ones_f = wp.tile([128, 128], F32)
        nc.vector.memset(ones_f, 1.0)
        nc.vector.tensor_copy(ones_sb, ones_f)
        make_identity(nc, ident_sb)

        wq3 = wqkv.rearrange("(k p) d -> p k d", p=128)
        x3 = xT.rearrange("(k p) s -> p k s", p=128)
        # x chunk (t=0, kg=0) first so the first matmul starts almost
        # immediately; weight k-tiles follow in per-tile DMAs
        w_sb = [wp.tile([128, DOUT], F32R, name=f"w{k}", tag=f"w{k}")
                for k in range(KT)]
        xg0 = xp.tile([128, KG, TCH], F32R, name="xg")
        nc.sync.dma_start(out=xg0, in_=x3[:, 0:KG, 0:TCH])
        for k in range(KT):
            nc.sync.dma_start(out=w_sb[k], in_=wq3[:, k, :])
        nc.sync.dma_start(out=cos_sb, in_=cos_t)
        nc.sync.dma_start(out=sin_sb, in_=sin_t)
        for t in range(NTCH):
            ps = [pp1.tile([128, TCH], F32, name=f"p1_{m}", tag=f"p1_{m}")
                  for m in range(MT)]
            for kg in range(KT // KG):
                if t == 0 and kg == 0:
                    xg = xg0
                else:
                    xg = xp.tile([128, KG, TCH], F32R, name="xg")
                    nc.sync.dma_start(
                        out=xg,
                        in_=x3[:, kg * KG:(kg + 1) * KG, t * TCH:(t + 1) * TCH])
                for ki in range(KG):
                    k = kg * KG + ki
                    for m in range(MT):
                        nc.tensor.matmul(
                            ps[m],
                            lhsT=w_sb[k][:, m * 128:(m + 1) * 128],
                            rhs=xg[:, ki, :],
                            start=(k == 0), stop=(k == KT - 1),
                        )
            cs = cos_sb[:, t * TCH:(t + 1) * TCH]
            sn = sin_sb[:, t * TCH:(t + 1) * TCH]
            for h in range(QH):
                _rope(nc, rp, ps[h], qT[:, h, t * TCH:(t + 1) * TCH], cs, sn)
            _rope(nc, rp, ps[QH], kT[:, t * TCH:(t + 1) * TCH], cs, sn)
            nc.scalar.copy(out=vT[:, t * TCH:(t + 1) * TCH], in_=ps[QH + 1])
            # V = vT.T for this chunk's kv tiles (PE transpose [d,tok]->[tok,d])
            for j in range(4 * t, 4 * t + 4):
                pv = pp1.tile([128, 128], F32, name="pvt", tag="pvt")
                nc.tensor.transpose(pv, vT[:, j * 128:(j + 1) * 128], ident_sb)
                nc.scalar.copy(out=V[:, j, :], in_=pv)

    # ---- phases 2+3: attention heads with per-head AllGather; o_proj for
    # head 0 interleaved into attention head 3, rest at the tail ---------
    with (
        tc.tile_pool(name="pt", bufs=8) as ptp,
        tc.tile_pool(name="ao", bufs=2) as aop,
        tc.tile_pool(name="ow", bufs=16) as owp,
        tc.tile_pool(name="at", bufs=2) as atp,
        tc.tile_pool(name="acc", bufs=1) as accp,
        tc.tile_pool(name="p2sc", bufs=2, space="PSUM") as pp2,
        tc.tile_pool(name="p2acc", bufs=1, space="PSUM") as pa2,
        tc.tile_pool(name="p3ps", bufs=1, space="PSUM") as pp3,
    ):
        # o_proj output accumulator: acc[:, b, :] = out rows b*128:(b+1)*128
        acc = accp.tile([128, S // 128, TCH], F32)
        ow3 = ow.rearrange("(k p) d -> p k d", p=128)
        ag3 = [attn_gat[h].rearrange("(r p) s -> p r s", p=128)
               for h in range(QH)]

        def attention_chunk(h, c):
            """One (head, q-chunk): scores+exp in j-pair waves, then PV."""
            jmax = 4 * c + 3
            po = pa2.tile([128, TCH], F32, name="po", tag="po")
            psum_s = pa2.tile([128, TCH], F32, name="ps", tag="ps")
            qslice = qT[:, h, c * TCH:(c + 1) * TCH]
            npair = (jmax + 1) // 2
            PW = 4  # j-pairs per pass-A/B wave (bounds live pt tiles)
            for p0 in range(0, npair, PW):
                p1 = min(p0 + PW, npair)
                pts = []
                for p in range(p0, p1):
                    # two score matmuls into one 2-bank psum tile, one exp
                    sc = pp2.tile([128, 2, TCH], F32, name="sc", tag="sc")
                    for i in range(2):
                        j = 2 * p + i
                        nc.tensor.matmul(
                            sc[:, i, :], lhsT=kT[:, j * 128:(j + 1) * 128],
                            rhs=qslice, start=True, stop=True)
                    pt = ptp.tile([128, 2, TCH], F32R, name="pt", tag="pt")
                    nc.scalar.activation(pt, sc, EXP, scale=SCALE)
                    for i in range(2):
                        j = 2 * p + i
                        rdiag = j - 4 * c
                        if rdiag >= 0:  # tile touches the causal diagonal
                            off = 384 - rdiag * 128
                            nc.vector.tensor_mul(
                                pt[:, i, :], pt[:, i, :],
                                stair_sb[:, off:off + TCH])
                    pts.append(pt)
                for idx, p in enumerate(range(p0, p1)):
                    for i in range(2):
                        j = 2 * p + i
                        nc.tensor.matmul(po, lhsT=V[:, j, :],
                                         rhs=pts[idx][:, i, :],
                                         start=(j == 0), stop=(j == jmax))
                        nc.tensor.matmul(psum_s, lhsT=ones_sb,
                                         rhs=pts[idx][:, i, :],
                                         start=(j == 0), stop=(j == jmax))
            rec = aop.tile([128, TCH], F32, name="rec")
            nc.vector.reciprocal(rec, psum_s)
            ao = aop.tile([128, TCH], F32, name="ao")
            nc.vector.tensor_mul(ao, po, rec)
            nc.sync.dma_start(
                out=attn_loc[h][:, c * TCH:(c + 1) * TCH], in_=ao)

        def allgather_head(h):
            nc.gpsimd.collective_compute(
                "AllGather",
                mybir.AluOpType.bypass,
                ins=[attn_loc[h][:, :]],
                outs=[attn_gat[h][:, :]],
                replica_groups=[list(range(NCORES))],
            )

        def oproj_load_weights(h):
            ows = []
            for r in range(NCORES):
                owk = owp.tile([128, DQ], F32R, name="owk", tag="owk")
                nc.sync.dma_start(out=owk, in_=ow3[:, r * QH + h, :])
                ows.append(owk)
            return ows

        def oproj_chunk(h, g, ows):
            """acc[:, 4g:4g+4, :] += sum_r at(r, h) @ ow(r, h) for 512 toks."""
            at = atp.tile([128, NCORES, TCH], F32R, name="at", tag="at")
            nc.sync.dma_start(
                out=at, in_=ag3[h][:, :, g * TCH:(g + 1) * TCH].bitcast(F32R))
            for mp in range(2):
                pcs = [pp3.tile([128, TCH], F32, name=f"pc{i}", tag=f"pc{i}")
                       for i in range(2)]
                for r in range(NCORES):
                    for i, mi in enumerate((2 * mp, 2 * mp + 1)):
                        nc.tensor.matmul(
                            pcs[i],
                            lhsT=at[:, r, mi * 128:(mi + 1) * 128],
                            rhs=ows[r],
                            start=(r == 0), stop=(r == NCORES - 1),
                        )
                for i, mi in enumerate((2 * mp, 2 * mp + 1)):
                    b = g * 4 + mi
                    if h == 0:
                        nc.scalar.copy(out=acc[:, b, :], in_=pcs[i])
                    else:
                        nc.vector.tensor_add(acc[:, b, :], acc[:, b, :],
                                             pcs[i])

        for h in range(3):
            for c in range(NTCH):
                attention_chunk(h, c)
            allgather_head(h)
        # head 3 attention interleaved with o_proj of the gathered head 0
        ows0 = oproj_load_weights(0)
        for c in range(NTCH):
            attention_chunk(3, c)
            oproj_chunk(0, c, ows0)
        allgather_head(3)
        for h in range(1, QH):
            ows = oproj_load_weights(h)
            for g in range(S // TCH):
                oproj_chunk(h, g, ows)

        nc.sync.dma_start(out=out.rearrange("(b p) d -> p b d", p=128), in_=acc)


_NC_CACHE = None


def build_program():
    global _NC_CACHE
    if _NC_CACHE is not None:
        return _NC_CACHE
    nc = bacc.Bacc("TRN2", target_bir_lowering=False, debug=False,
                   num_devices=NCORES)
    ins = {
        "xT": nc.dram_tensor("xT", [HIDDEN, S], F32R, kind="ExternalInput").ap(),
        "wqkv": nc.dram_tensor("wqkv", [HIDDEN, DOUT], F32R,
                               kind="ExternalInput").ap(),
        "ow": nc.dram_tensor("ow", [HIDDEN, DQ], F32R, kind="ExternalInput").ap(),
        "cos_t": nc.dram_tensor("cos_t", [64, S], F32, kind="ExternalInput").ap(),
        "sin_t": nc.dram_tensor("sin_t", [64, S], F32, kind="ExternalInput").ap(),
        "stair": nc.dram_tensor("stair", [128, 896], F32,
                                kind="ExternalInput").ap(),
    }
    outs = {"out": nc.dram_tensor("out", [S, DQ], F32, kind="ExternalOutput").ap()}
    with tile.TileContext(nc) as tc:
        with ExitStack() as ctx:
            build_kernel_body(ctx, tc, outs, ins)
    nc.compile()
    _NC_CACHE = nc
    return nc


def make_in_maps(hidden_states, position_ids, q_w, k_w, v_w, o_w):
    x = np.asarray(hidden_states, dtype=np.float32).reshape(S, HIDDEN)
    xT = np.ascontiguousarray(x.T)
    pos = np.asarray(position_ids).reshape(S).astype(np.float64)
    inv = 1.0 / (THETA ** (np.arange(0, HD, 2, dtype=np.float64) / HD))
    fr = inv[:, None] * pos[None, :]                       # [64, S]
    cos_t = np.cos(fr).astype(np.float32)
    sin_t = np.sin(fr).astype(np.float32)
    u = np.arange(896, dtype=np.int64)[None, :]
    kvi = np.arange(128, dtype=np.int64)[:, None]
    stair = ((u - kvi) >= 384).astype(np.float32)          # [128, 896]

    q_w = np.asarray(q_w, dtype=np.float32)
    k_w = np.asarray(k_w, dtype=np.float32)
    v_w = np.asarray(v_w, dtype=np.float32)
    o_w = np.asarray(o_w, dtype=np.float32)

    in_maps = []
    for c in range(NCORES):
        wqkv = np.ascontiguousarray(np.concatenate(
            [q_w[:, c * DQ:(c + 1) * DQ],
             k_w[:, c * HD:(c + 1) * HD],
             v_w[:, c * HD:(c + 1) * HD]], axis=1))
        owc = np.ascontiguousarray(o_w[:, c * DQ:(c + 1) * DQ])
        in_maps.append({"xT": xT, "wqkv": wqkv, "ow": owc,
                        "cos_t": cos_t, "sin_t": sin_t, "stair": stair})
    return in_maps


def run(inputs: dict, trace: bool = False):
    """Run on the 8 NeuronCores; returns (full_output, BassKernelResults)."""
    nc = build_program()
    in_maps = make_in_maps(**inputs)
    res = run_bass_kernel_spmd(nc, in_maps, core_ids=list(range(NCORES)),
                               trace=trace)
    full = np.concatenate([res.results[c]["out"] for c in range(NCORES)], axis=1)
    return full.reshape(1, S, HIDDEN), res


def kernel(**inputs) -> np.ndarray:
    out, _ = run(inputs)
    return out



# revision 8
# speedup vs baseline: 1.0628x; 1.0628x over previous
# Mistral sliding-window attention (B=1, S=2048, H=4096, 32 q heads / 8 kv
# heads, window 4096 -> plain causal at this S) on 8 Trainium2 NeuronCores.
#
# Sharding: tensor-parallel over heads. Core c owns q heads 4c..4c+3 and kv
# head c; hidden_states replicated (host-transposed to [H, S] bf16).
#
# v2 design (dense-PE pipeline, bf16 data):
# - Phase A: QKV projection in 4 token chunks of 512, m-pairs accumulated in
#   three 2-bank PSUM tags; psum->SBUF staging copies on ACT free the banks
#   within ~1us so consecutive chunks never stall; RoPE runs on DVE from the
#   bf16 staging during the next chunk; V tiles are PE-transposed between
#   k-groups of the next chunk (off the critical path).
# - Phase B: per q-chunk c: attention for all 4 heads (scores in 3 rotating
#   2-bank psum tags so the PE never waits on the ACT exp; probabilities in
#   bf16; causal staircase mask multiplied on DVE; PV + ones-denominator
#   accumulate in one shared tag), then immediately the o_proj for token
#   block g=c: contributions of all 4 local heads accumulate IN PSUM
#   (no DVE adds), partial [512, 4096] written to DRAM in bf16, and a
#   ReduceScatter(add) over the 8 cores fires per block so collectives
#   overlap the next chunk's attention. Host stitches the 8x4x64-row
#   outputs.

from contextlib import ExitStack

import numpy as np
import ml_dtypes

import concourse.bacc as bacc
import concourse.bass as bass
import concourse.mybir as mybir
import concourse.tile as tile
from concourse.bass_utils import run_bass_kernel_spmd
from concourse.masks import make_identity

HIDDEN = 4096
NH = 32
NKV = 8
HD = 128
THETA = 10000.0
S = 2048
NCORES = 8

QH = NH // NCORES          # 4 q heads per core
DQ = QH * HD               # 512 (per-core q/attn width)
DOUT = DQ + 2 * HD         # 768 = q heads + k + v projection width
KT = HIDDEN // 128         # 32 contraction tiles
KG = 8                     # k-tiles per x DMA / inner k-group
TCH = 512                  # token chunk (matmul moving dim)
NTCH = S // TCH            # 4
KVT = S // 128             # 16 kv tiles
SCALE = 1.0 / float(np.sqrt(HD))

F32 = mybir.dt.float32
BF16 = mybir.dt.bfloat16
EXP = mybir.ActivationFunctionType.Exp
NPBF16 = ml_dtypes.bfloat16


def _rope(nc, rp, stg_half, qdst, cs2, sn2):
    """RoPE one [128, 512] head-tile: stg (bf16 SBUF) -> qdst (bf16 SBUF).

    qdst = stg*cs2 + rotate_half(stg)*sn2, with cs2 = [cos; cos] and
    sn2 = [-sin; sin] stacked on 128 partitions (host-precomputed), so all
    DVE ops are partition-aligned; the rotate is two SBUF->SBUF DMAs.
    """
    b = rp.tile([128, TCH], BF16, name="rope_b")
    nc.sync.dma_start(out=b[0:64, :], in_=stg_half[64:128, :])
    nc.sync.dma_start(out=b[64:128, :], in_=stg_half[0:64, :])
    ta = rp.tile([128, TCH], BF16, name="rope_t")
    tb = rp.tile([128, TCH], BF16, name="rope_u")
    nc.vector.tensor_mul(ta, stg_half, cs2)
    nc.vector.tensor_mul(tb, b, sn2)
    nc.vector.tensor_add(qdst, ta, tb)


def build_kernel_body(ctx: ExitStack, tc: tile.TileContext, outs, ins):
    nc = tc.nc
    xT, wqkv, ow, cos_t, sin_t, stair = (
        ins["xT"], ins["wqkv"], ins["ow"], ins["cos_t"], ins["sin_t"], ins["stair"],
    )
    out = outs["out"]

    partial = nc.dram_tensor("partial", [S, HIDDEN], BF16).ap()
    rs_out = nc.dram_tensor("rs_out", [NTCH, S // NTCH // NCORES, HIDDEN],
                            BF16).ap()
    p3 = partial.rearrange("(b p) d -> p b d", p=128)

    singles = ctx.enter_context(tc.tile_pool(name="singles", bufs=1))
    # persistent bf16 state
    qT = singles.tile([128, QH, S], BF16)     # roped q, head h -> qT[:, h, :]
    kT = singles.tile([128, S], BF16)         # roped k
    V = singles.tile([128, KVT, HD], BF16)    # V[:, j, :] = [tok 128, d 128]
    ao = singles.tile([128, QH, S], BF16)     # attention out per head
    ones_sb = singles.tile([128, 128], BF16)
    ident_sb = singles.tile([128, 128], BF16)
    stair_sb = singles.tile([128, 896], BF16)
    cos_sb = singles.tile([128, S], BF16)
    sin_sb = singles.tile([128, S], BF16)
    ow_sb = singles.tile([128, QH, HIDDEN], BF16)   # o_w rows, d on partition

    # shared PSUM tags: three [128, 2, 512] f32 accumulators (6 banks)
    psh = ctx.enter_context(tc.tile_pool(name="psh", bufs=1, space="PSUM"))

    def T(i):
        return psh.tile([128, 2, TCH], F32, name=f"T{i}", tag=f"T{i}")

    # ---- DMA front matter ------------------------------------------------
    wq3 = wqkv.rearrange("(k p) d -> p k d", p=128)
    x3 = xT.rearrange("(k p) s -> p k s", p=128)

    with (
        tc.tile_pool(name="wq", bufs=1) as wp,
        tc.tile_pool(name="xt", bufs=3) as xp,
        tc.tile_pool(name="stg", bufs=6) as sp,
        tc.tile_pool(name="rope", bufs=4) as rp,
        tc.tile_pool(name="pva", bufs=1, space="PSUM") as ppv,
    ):
        xg0 = xp.tile([128, KG, TCH], BF16, name="xg")
        nc.sync.dma_start(out=xg0, in_=x3[:, 0:KG, 0:TCH])
        w_sb = [wp.tile([128, DOUT], BF16, name=f"w{k}", tag=f"w{k}")
                for k in range(KT)]
        for k in range(KT):
            nc.sync.dma_start(out=w_sb[k], in_=wq3[:, k, :])
        nc.sync.dma_start(out=cos_sb, in_=cos_t)
        nc.sync.dma_start(out=sin_sb, in_=sin_t)
        nc.sync.dma_start(out=stair_sb, in_=stair)
        nc.sync.dma_start(out=ow_sb, in_=ow)
        nc.vector.memset(ones_sb, 1.0)
        make_identity(nc, ident_sb)

        # ---- phase A: QKV projection + RoPE ------------------------------
        # stg tiles of previous chunk, consumed while the next chunk runs
        pend = None  # (chunk, [stg0, stg1, stg2])

        def drain_pending(after_kg):
            """Emit rope/transpose work for the previous chunk, interleaved
            into the current chunk's PE stream (after k-group `after_kg`)."""
            if pend is None:
                return
            c, stgs = pend
            lo = c * TCH
            cs = cos_sb[:, lo:lo + TCH]
            sn = sin_sb[:, lo:lo + TCH]
            if after_kg == 1:
                for mg in range(2):
                    for i in range(2):
                        h = 2 * mg + i
                        _rope(nc, rp, stgs[mg][:, i, :],
                              qT[:, h, lo:lo + TCH], cs, sn)
                _rope(nc, rp, stgs[2][:, 0, :], kT[:, lo:lo + TCH], cs, sn)
            elif after_kg in (2, 3):
                # two V transposes per k-group seam (PE, cheap, late-emitted)
                for t in range(2 * (after_kg - 2), 2 * (after_kg - 1)):
                    j = 4 * c + t
                    pv = ppv.tile([128, 128], BF16, name="pv", tag="pv")
                    nc.tensor.transpose(
                        pv, stgs[2][:, 1, t * 128:(t + 1) * 128], ident_sb)
                    nc.scalar.copy(out=V[:, j, :], in_=pv)

        for c in range(NTCH):
            lo = c * TCH
            pss = [T(mg) for mg in range(3)]
            for kg in range(KT // KG):
                if c == 0 and kg == 0:
                    xg = xg0
                else:
                    xg = xp.tile([128, KG, TCH], BF16, name="xg")
                    nc.sync.dma_start(
                        out=xg,
                        in_=x3[:, kg * KG:(kg + 1) * KG, lo:lo + TCH])
                for mg in range(3):
                    for ki in range(KG):
                        k = kg * KG + ki
                        for i in range(2):
                            m = 2 * mg + i
                            nc.tensor.matmul(
                                pss[mg][:, i, :],
                                lhsT=w_sb[k][:, m * 128:(m + 1) * 128],
                                rhs=xg[:, ki, :],
                                start=(k == 0), stop=(k == KT - 1),
                            )
                drain_pending(kg)
            # copy psum -> bf16 staging (ACT), freeing banks for next chunk
            stgs = []
            for mg in range(3):
                stg = sp.tile([128, 2, TCH], BF16, name=f"stg{mg}")
                nc.scalar.copy(out=stg, in_=pss[mg])
                stgs.append(stg)
            pend = (c, stgs)
        # drain the last chunk's rope + V transposes
        for kg in range(1, 4):
            drain_pending(kg)
        pend = None

    # ---- phase B: attention + o_proj + ReduceScatter ---------------------
    with (
        tc.tile_pool(name="pt", bufs=6) as ptp,
        tc.tile_pool(name="nrm", bufs=2) as nrmp,
        tc.tile_pool(name="ost", bufs=4) as ostp,
        tc.tile_pool(name="psb", bufs=1, space="PSUM") as psb,
    ):
        def attention_chunk(h, c):
            jmax = 4 * c + 3
            npair = 2 * c + 2
            t2 = T(2)
            po = t2[:, 0, :]
            ps = t2[:, 1, :]
            qslice = qT[:, h, c * TCH:(c + 1) * TCH]
            scs = {}
            pts = {}

            def sc_wave(p):
                if p % 3 < 2:
                    sct = T(p % 3)
                else:
                    sct = psb.tile([128, 2, TCH], F32, name="T3", tag="T3")
                for i in range(2):
                    j = 2 * p + i
                    nc.tensor.matmul(
                        sct[:, i, :], lhsT=kT[:, j * 128:(j + 1) * 128],
                        rhs=qslice, start=True, stop=True)
                scs[p] = sct
                pt = ptp.tile([128, 2, TCH], BF16, name="pt")
                nc.scalar.activation(pt, sct, EXP, scale=SCALE)
                for i in range(2):
                    j = 2 * p + i
                    rdiag = j - 4 * c
                    if rdiag >= 0:  # tile touches the causal diagonal
                        off = 384 - rdiag * 128
                        nc.vector.tensor_mul(
                            pt[:, i, :], pt[:, i, :],
                            stair_sb[:, off:off + TCH])
                pts[p] = pt

            def pv_wave(p):
                for i in range(2):
                    j = 2 * p + i
                    nc.tensor.matmul(po, lhsT=V[:, j, :],
                                     rhs=pts[p][:, i, :],
                                     start=(j == 0), stop=(j == jmax))
                    nc.tensor.matmul(ps, lhsT=ones_sb,
                                     rhs=pts[p][:, i, :],
                                     start=(j == 0), stop=(j == jmax))

            # run 2 score waves ahead of the PV waves so the PE never waits
            # for the ACT exp
            for p in range(npair):
                sc_wave(p)
                if p >= 2:
                    pv_wave(p - 2)
            for p in range(max(0, npair - 2), npair):
                pv_wave(p)

            rec = nrmp.tile([128, TCH], F32, name="rec")
            nc.vector.reciprocal(rec, ps)
            nc.vector.tensor_mul(ao[:, h, c * TCH:(c + 1) * TCH], po, rec)

        def oproj_chunk(g):
            """partial[g*512:(g+1)*512, :] = local 4-head o_proj block."""
            for cc in range(HIDDEN // TCH):
                acc = [T(0), T(1)]
                for h in range(QH):
                    for t in range(4):
                        nc.tensor.matmul(
                            acc[t // 2][:, t % 2, :],
                            lhsT=ao[:, h, g * TCH + t * 128:
                                    g * TCH + (t + 1) * 128],
                            rhs=ow_sb[:, h, cc * TCH:(cc + 1) * TCH],
                            start=(h == 0), stop=(h == QH - 1),
                        )
                for half in range(2):
                    ost = ostp.tile([128, 2, TCH], BF16, name="ost")
                    if half == 0:
                        nc.scalar.copy(out=ost, in_=acc[0])
                    else:
                        nc.vector.tensor_copy(ost, acc[1])
                    nc.sync.dma_start(
                        out=p3[:, g * 4 + 2 * half:g * 4 + 2 * half + 2,
                               cc * TCH:(cc + 1) * TCH],
                        in_=ost)

        for c in range(NTCH):
            for h in range(QH):
                attention_chunk(h, c)
            oproj_chunk(c)
            nc.gpsimd.collective_compute(
                "ReduceScatter",
                mybir.AluOpType.add,
                ins=[partial[c * TCH:(c + 1) * TCH, :]],
                outs=[rs_out[c]],
                replica_groups=[list(range(NCORES))],
            )
            nc.sync.dma_start(out=out[c], in_=rs_out[c])


_NC_CACHE = None


def build_program():
    global _NC_CACHE
    if _NC_CACHE is not None:
        return _NC_CACHE
    nc = bacc.Bacc("TRN2", target_bir_lowering=False, debug=False,
                   num_devices=NCORES)
    ins = {
        "xT": nc.dram_tensor("xT", [HIDDEN, S], BF16, kind="ExternalInput").ap(),
        "wqkv": nc.dram_tensor("wqkv", [HIDDEN, DOUT], BF16,
                               kind="ExternalInput").ap(),
        "ow": nc.dram_tensor("ow", [128, QH, HIDDEN], BF16,
                             kind="ExternalInput").ap(),
        "cos_t": nc.dram_tensor("cos_t", [128, S], BF16, kind="ExternalInput").ap(),
        "sin_t": nc.dram_tensor("sin_t", [128, S], BF16, kind="ExternalInput").ap(),
        "stair": nc.dram_tensor("stair", [128, 896], BF16,
                                kind="ExternalInput").ap(),
    }
    outs = {"out": nc.dram_tensor(
        "out", [NTCH, S // NTCH // NCORES, HIDDEN], BF16,
        kind="ExternalOutput").ap()}
    with tile.TileContext(nc) as tc:
        with ExitStack() as ctx:
            build_kernel_body(ctx, tc, outs, ins)
    nc.compile()
    _NC_CACHE = nc
    return nc


def make_in_maps(hidden_states, position_ids, q_w, k_w, v_w, o_w):
    x = np.asarray(hidden_states, dtype=np.float32).reshape(S, HIDDEN)
    xT = np.ascontiguousarray(x.T).astype(NPBF16)
    pos = np.asarray(position_ids).reshape(S).astype(np.float64)
    inv = 1.0 / (THETA ** (np.arange(0, HD, 2, dtype=np.float64) / HD))
    fr = inv[:, None] * pos[None, :]                       # [64, S]
    cos_t = np.concatenate([np.cos(fr), np.cos(fr)], 0).astype(NPBF16)
    sin_t = np.concatenate([-np.sin(fr), np.sin(fr)], 0).astype(NPBF16)
    u = np.arange(896, dtype=np.int64)[None, :]
    kvi = np.arange(128, dtype=np.int64)[:, None]
    stair = ((u - kvi) >= 384).astype(NPBF16)              # [128, 896]

    q_w = np.asarray(q_w, dtype=np.float32)
    k_w = np.asarray(k_w, dtype=np.float32)
    v_w = np.asarray(v_w, dtype=np.float32)
    o_w = np.asarray(o_w, dtype=np.float32)

    in_maps = []
    for c in range(NCORES):
        wqkv = np.ascontiguousarray(np.concatenate(
            [q_w[:, c * DQ:(c + 1) * DQ],
             k_w[:, c * HD:(c + 1) * HD],
             v_w[:, c * HD:(c + 1) * HD]], axis=1)).astype(NPBF16)
        owc = np.ascontiguousarray(
            o_w[c * DQ:(c + 1) * DQ, :].reshape(QH, 128, HIDDEN)
            .transpose(1, 0, 2)).astype(NPBF16)
        in_maps.append({"xT": xT, "wqkv": wqkv, "ow": owc,
                        "cos_t": cos_t, "sin_t": sin_t, "stair": stair})
    return in_maps


def assemble_output(outs_per_core):
    """outs_per_core[c] = [NTCH, 64, HIDDEN] bf16; stitch to [1, S, HIDDEN]."""
    rows = S // NTCH // NCORES
    full = np.empty((S, HIDDEN), dtype=np.float32)
    for c in range(NCORES):
        o = np.asarray(outs_per_core[c]).astype(np.float32)
        for g in range(NTCH):
            r0 = g * TCH + c * rows
            full[r0:r0 + rows] = o[g]
    return full.reshape(1, S, HIDDEN)


def run(inputs: dict, trace: bool = False):
    """Run on the 8 NeuronCores; returns (full_output, BassKernelResults)."""
    nc = build_program()
    in_maps = make_in_maps(**inputs)
    res = run_bass_kernel_spmd(nc, in_maps, core_ids=list(range(NCORES)),
                               trace=trace)
    full = assemble_output([res.results[c]["out"] for c in range(NCORES)])
    return full, res


def kernel(**inputs) -> np.ndarray:
    out, _ = run(inputs)
    return out


# revision 16
# speedup vs baseline: 1.1094x; 1.0439x over previous
# Mistral sliding-window attention (B=1, S=2048, H=4096, 32 q heads / 8 kv
# heads, window 4096 -> plain causal at this S) on 8 Trainium2 NeuronCores.
#
# Sharding: tensor-parallel over heads. Core c owns q heads 4c..4c+3 and kv
# head c; hidden_states replicated (host-transposed to [H, S] bf16).
#
# v2 design (dense-PE pipeline, bf16 data):
# - Phase A: QKV projection in 4 token chunks of 512, m-pairs accumulated in
#   three 2-bank PSUM tags; psum->SBUF staging copies on ACT free the banks
#   within ~1us so consecutive chunks never stall; RoPE runs on DVE from the
#   bf16 staging during the next chunk; V tiles are PE-transposed between
#   k-groups of the next chunk (off the critical path).
# - Phase B: per q-chunk c: attention for all 4 heads (scores in 3 rotating
#   2-bank psum tags so the PE never waits on the ACT exp; probabilities in
#   bf16; causal staircase mask multiplied on DVE; PV + ones-denominator
#   accumulate in one shared tag), then immediately the o_proj for token
#   block g=c: contributions of all 4 local heads accumulate IN PSUM
#   (no DVE adds), partial [512, 4096] written to DRAM in bf16, and a
#   ReduceScatter(add) over the 8 cores fires per block so collectives
#   overlap the next chunk's attention. Host stitches the 8x4x64-row
#   outputs.

from contextlib import ExitStack

import numpy as np
import ml_dtypes

import concourse.bacc as bacc
import concourse.bass as bass
import concourse.mybir as mybir
import concourse.tile as tile
from concourse.bass_utils import run_bass_kernel_spmd
from concourse.masks import make_identity

HIDDEN = 4096
NH = 32
NKV = 8
HD = 128
THETA = 10000.0
S = 2048
NCORES = 8

QH = NH // NCORES          # 4 q heads per core
DQ = QH * HD               # 512 (per-core q/attn width)
DOUT = DQ + 2 * HD         # 768 = q heads + k + v projection width
KT = HIDDEN // 128         # 32 contraction tiles
KG = 8                     # k-tiles per x DMA / inner k-group
TCH = 512                  # token chunk (matmul moving dim)
NTCH = S // TCH            # 4
KVT = S // 128             # 16 kv tiles
SCALE = 1.0 / float(np.sqrt(HD))

F32 = mybir.dt.float32
BF16 = mybir.dt.bfloat16
EXP = mybir.ActivationFunctionType.Exp
NPBF16 = ml_dtypes.bfloat16


def _rope(nc, rp, stg_half, qdst, cs2, sn2):
    """RoPE one [128, 512] head-tile: stg (bf16 SBUF) -> qdst (bf16 SBUF).

    qdst = stg*cs2 + rotate_half(stg)*sn2, with cs2 = [cos; cos] and
    sn2 = [-sin; sin] stacked on 128 partitions (host-precomputed), so all
    DVE ops are partition-aligned; the rotate is two SBUF->SBUF DMAs.
    """
    b = rp.tile([128, TCH], BF16, name="rope_b")
    nc.sync.dma_start(out=b[0:64, :], in_=stg_half[64:128, :])
    nc.sync.dma_start(out=b[64:128, :], in_=stg_half[0:64, :])
    ta = rp.tile([128, TCH], BF16, name="rope_t")
    tb = rp.tile([128, TCH], BF16, name="rope_u")
    nc.vector.tensor_mul(ta, stg_half, cs2)
    nc.vector.tensor_mul(tb, b, sn2)
    nc.vector.tensor_add(qdst, ta, tb)


def build_kernel_body(ctx: ExitStack, tc: tile.TileContext, outs, ins):
    nc = tc.nc
    xT, wqkv, ow, cos_t, sin_t, stair = (
        ins["xT"], ins["wqkv"], ins["ow"], ins["cos_t"], ins["sin_t"], ins["stair"],
    )
    out = outs["out"]

    # one partial tensor per token block so RS(g) never false-serializes
    # against the o_proj writes of block g+1
    partials = [nc.dram_tensor(f"partial{g}", [TCH, HIDDEN], BF16).ap()
                for g in range(NTCH)]
    rs_out = nc.dram_tensor("rs_out", [NTCH, S // NTCH // NCORES, HIDDEN],
                            BF16).ap()
    p3s = [p.rearrange("(b p) d -> p b d", p=128) for p in partials]

    singles = ctx.enter_context(tc.tile_pool(name="singles", bufs=1))
    # persistent bf16 state
    qT = singles.tile([128, QH, S], BF16)     # roped q, head h -> qT[:, h, :]
    kT = singles.tile([128, S], BF16)         # roped k
    V = singles.tile([128, KVT, HD], BF16)    # V[:, j, :] = [tok 128, d 128]
    ao = singles.tile([128, QH, S], BF16)     # attention out per head
    ones_sb = singles.tile([128, 128], BF16)
    ident_sb = singles.tile([128, 128], BF16)
    stair_sb = singles.tile([128, 896], BF16)
    cos_sb = singles.tile([128, S], BF16)
    sin_sb = singles.tile([128, S], BF16)
    ow_sb = singles.tile([128, QH, HIDDEN], BF16)   # o_w rows, d on partition

    # shared PSUM tags: three [128, 2, 512] f32 accumulators (6 banks)
    psh = ctx.enter_context(tc.tile_pool(name="psh", bufs=1, space="PSUM"))

    def T(i):
        return psh.tile([128, 2, TCH], F32, name=f"T{i}", tag=f"T{i}")

    # ---- DMA front matter ------------------------------------------------
    wq3 = wqkv.rearrange("(k p) d -> p k d", p=128)
    x3 = xT.rearrange("(k p) s -> p k s", p=128)

    with (
        tc.tile_pool(name="wq", bufs=1) as wp,
        tc.tile_pool(name="xt", bufs=4) as xp,
        tc.tile_pool(name="stg", bufs=6) as sp,
        tc.tile_pool(name="rope", bufs=3) as rp,
        tc.tile_pool(name="pva", bufs=1, space="PSUM") as ppv,
    ):
        # chunk-0 x tiles interleaved with the weight k-groups so the PE's
        # k-ordered consumption is never starved by a long weight queue
        w_sb = [wp.tile([128, DOUT], BF16, name=f"w{k}", tag=f"w{k}")
                for k in range(KT)]
        xg_c0 = []
        for kg in range(KT // KG):
            xg = xp.tile([128, KG, TCH], BF16, name="xg")
            nc.sync.dma_start(out=xg, in_=x3[:, kg * KG:(kg + 1) * KG, 0:TCH])
            xg_c0.append(xg)
            for k in range(kg * KG, (kg + 1) * KG):
                nc.sync.dma_start(out=w_sb[k], in_=wq3[:, k, :])
        nc.sync.dma_start(out=cos_sb, in_=cos_t)
        nc.sync.dma_start(out=sin_sb, in_=sin_t)
        nc.sync.dma_start(out=stair_sb, in_=stair)
        nc.vector.memset(ones_sb, 1.0)
        make_identity(nc, ident_sb)

        # ---- phase A: QKV projection + RoPE ------------------------------
        # stg tiles of previous chunk, consumed while the next chunk runs
        pend = None  # (chunk, [stg0, stg1, stg2])

        def drain_pending(after_kg):
            """Emit rope/transpose work for the previous chunk, interleaved
            into the current chunk's PE stream (after k-group `after_kg`)."""
            if pend is None:
                return
            c, stgs = pend
            lo = c * TCH
            cs = cos_sb[:, lo:lo + TCH]
            sn = sin_sb[:, lo:lo + TCH]
            if after_kg == 1:
                for mg in range(2):
                    for i in range(2):
                        h = 2 * mg + i
                        _rope(nc, rp, stgs[mg][:, i, :],
                              qT[:, h, lo:lo + TCH], cs, sn)
                _rope(nc, rp, stgs[2][:, 0, :], kT[:, lo:lo + TCH], cs, sn)
            elif after_kg in (2, 3):
                # two V transposes per k-group seam (PE, cheap, late-emitted)
                for t in range(2 * (after_kg - 2), 2 * (after_kg - 1)):
                    j = 4 * c + t
                    pv = ppv.tile([128, 128], BF16, name="pv", tag="pv")
                    nc.tensor.transpose(
                        pv, stgs[2][:, 1, t * 128:(t + 1) * 128], ident_sb)
                    nc.scalar.copy(out=V[:, j, :], in_=pv)

        for c in range(NTCH):
            lo = c * TCH
            pss = [T(mg) for mg in range(3)]
            for kg in range(KT // KG):
                if c == 0:
                    xg = xg_c0[kg]
                else:
                    xg = xp.tile([128, KG, TCH], BF16, name="xg")
                    nc.sync.dma_start(
                        out=xg,
                        in_=x3[:, kg * KG:(kg + 1) * KG, lo:lo + TCH])
                for mg in range(3):
                    for ki in range(KG):
                        k = kg * KG + ki
                        for i in range(2):
                            m = 2 * mg + i
                            nc.tensor.matmul(
                                pss[mg][:, i, :],
                                lhsT=w_sb[k][:, m * 128:(m + 1) * 128],
                                rhs=xg[:, ki, :],
                                start=(k == 0), stop=(k == KT - 1),
                            )
                drain_pending(kg)
            # copy psum -> bf16 staging (ACT), freeing banks for next chunk
            stgs = []
            for mg in range(3):
                stg = sp.tile([128, 2, TCH], BF16, name=f"stg{mg}")
                nc.scalar.copy(out=stg, in_=pss[mg])
                stgs.append(stg)
            pend = (c, stgs)
            if c == 0:
                # o_proj weights: 4.2MB, queued after chunk-0 supply so it
                # loads during chunks 1-2 without starving the projection
                nc.sync.dma_start(out=ow_sb, in_=ow)
        # drain the last chunk's rope + V transposes
        for kg in range(1, 4):
            drain_pending(kg)
        pend = None

    # ---- phase B: attention + o_proj + ReduceScatter ---------------------
    with (
        tc.tile_pool(name="pt", bufs=6) as ptp,
        tc.tile_pool(name="nrm", bufs=2) as nrmp,
        tc.tile_pool(name="ost", bufs=4) as ostp,
        tc.tile_pool(name="psb", bufs=1, space="PSUM") as psb,
    ):
        # score-psum tags rotate GLOBALLY across waves (not per head) so a
        # head's first scores reuse the oldest buffer - the one whose exp
        # finished ~2 waves ago - never the one still being consumed.
        wave_ctr = [0]

        def sc_tile(w):
            i = w % 3
            if i < 2:
                return T(i)
            return psb.tile([128, 2, TCH], F32, name="T3", tag="T3")

        def attention_chunk(h, c, pvq):
            jmax = 4 * c + 3
            npair = 2 * c + 2
            t2 = T(2)
            po = t2[:, 0, :]
            ps = t2[:, 1, :]
            qslice = qT[:, h, c * TCH:(c + 1) * TCH]

            def pv_wave(args):
                pt, p, po_, ps_ = args
                for i in range(2):
                    j = 2 * p + i
                    nc.tensor.matmul(po_, lhsT=V[:, j, :], rhs=pt[:, i, :],
                                     start=(j == 0), stop=(j == jmax))
                    nc.tensor.matmul(ps_, lhsT=ones_sb, rhs=pt[:, i, :],
                                     start=(j == 0), stop=(j == jmax))

            for p in range(npair):
                sct = sc_tile(wave_ctr[0])
                wave_ctr[0] += 1
                for i in range(2):
                    j = 2 * p + i
                    nc.tensor.matmul(
                        sct[:, i, :], lhsT=kT[:, j * 128:(j + 1) * 128],
                        rhs=qslice, start=True, stop=True)
                pt = ptp.tile([128, 2, TCH], BF16, name="pt")
                nc.scalar.activation(pt, sct, EXP, scale=SCALE)
                for i in range(2):
                    j = 2 * p + i
                    rdiag = j - 4 * c
                    if rdiag >= 0:  # tile touches the causal diagonal
                        off = 384 - rdiag * 128
                        nc.vector.tensor_mul(
                            pt[:, i, :], pt[:, i, :],
                            stair_sb[:, off:off + TCH])
                pvq.append((pv_wave, (pt, p, po, ps), None))
                if len(pvq) > 2:
                    fn, args, fin = pvq.pop(0)
                    fn(args)
                    if fin is not None:
                        fin()
            # normalize once this head's last PV retires from the lag queue
            def finalize(h=h, c=c, po=po, ps=ps):
                rec = nrmp.tile([128, TCH], F32, name="rec")
                nc.vector.reciprocal(rec, ps)
                nc.vector.tensor_mul(ao[:, h, c * TCH:(c + 1) * TCH], po, rec)
            fn, args, fin = pvq[-1]
            pvq[-1] = (fn, args, finalize)

        def drain_pvq(pvq):
            while pvq:
                fn, args, fin = pvq.pop(0)
                fn(args)
                if fin is not None:
                    fin()

        def oproj_chunk(g):
            """partials[g] = local 4-head o_proj for tokens [g*512,(g+1)*512)."""
            for cc in range(HIDDEN // TCH):
                acc = [T(0), T(1)]
                for h in range(QH):
                    for t in range(4):
                        nc.tensor.matmul(
                            acc[t // 2][:, t % 2, :],
                            lhsT=ao[:, h, g * TCH + t * 128:
                                    g * TCH + (t + 1) * 128],
                            rhs=ow_sb[:, h, cc * TCH:(cc + 1) * TCH],
                            start=(h == 0), stop=(h == QH - 1),
                        )
                for half in range(2):
                    ost = ostp.tile([128, 2, TCH], BF16, name="ost")
                    if half == 0:
                        nc.scalar.copy(out=ost, in_=acc[0])
                    else:
                        nc.vector.tensor_copy(ost, acc[1])
                    nc.sync.dma_start(
                        out=p3s[g][:, 2 * half:2 * half + 2,
                                   cc * TCH:(cc + 1) * TCH],
                        in_=ost)

        for c in range(NTCH):
            pvq = []
            for h in range(QH):
                attention_chunk(h, c, pvq)
            drain_pvq(pvq)
            oproj_chunk(c)
            nc.gpsimd.collective_compute(
                "ReduceScatter",
                mybir.AluOpType.add,
                ins=[partials[c][:, :]],
                outs=[rs_out[c]],
                replica_groups=[list(range(NCORES))],
            )
            nc.sync.dma_start(out=out[c], in_=rs_out[c])


_NC_CACHE = None


def build_program():
    global _NC_CACHE
    if _NC_CACHE is not None:
        return _NC_CACHE
    nc = bacc.Bacc("TRN2", target_bir_lowering=False, debug=False,
                   num_devices=NCORES)
    ins = {
        "xT": nc.dram_tensor("xT", [HIDDEN, S], BF16, kind="ExternalInput").ap(),
        "wqkv": nc.dram_tensor("wqkv", [HIDDEN, DOUT], BF16,
                               kind="ExternalInput").ap(),
        "ow": nc.dram_tensor("ow", [128, QH, HIDDEN], BF16,
                             kind="ExternalInput").ap(),
        "cos_t": nc.dram_tensor("cos_t", [128, S], BF16, kind="ExternalInput").ap(),
        "sin_t": nc.dram_tensor("sin_t", [128, S], BF16, kind="ExternalInput").ap(),
        "stair": nc.dram_tensor("stair", [128, 896], BF16,
                                kind="ExternalInput").ap(),
    }
    outs = {"out": nc.dram_tensor(
        "out", [NTCH, S // NTCH // NCORES, HIDDEN], BF16,
        kind="ExternalOutput").ap()}
    with tile.TileContext(nc) as tc:
        with ExitStack() as ctx:
            build_kernel_body(ctx, tc, outs, ins)
    nc.compile()
    _NC_CACHE = nc
    return nc


def make_in_maps(hidden_states, position_ids, q_w, k_w, v_w, o_w):
    x = np.asarray(hidden_states, dtype=np.float32).reshape(S, HIDDEN)
    xT = np.ascontiguousarray(x.T).astype(NPBF16)
    pos = np.asarray(position_ids).reshape(S).astype(np.float64)
    inv = 1.0 / (THETA ** (np.arange(0, HD, 2, dtype=np.float64) / HD))
    fr = inv[:, None] * pos[None, :]                       # [64, S]
    cos_t = np.concatenate([np.cos(fr), np.cos(fr)], 0).astype(NPBF16)
    sin_t = np.concatenate([-np.sin(fr), np.sin(fr)], 0).astype(NPBF16)
    u = np.arange(896, dtype=np.int64)[None, :]
    kvi = np.arange(128, dtype=np.int64)[:, None]
    stair = ((u - kvi) >= 384).astype(NPBF16)              # [128, 896]

    q_w = np.asarray(q_w, dtype=np.float32)
    k_w = np.asarray(k_w, dtype=np.float32)
    v_w = np.asarray(v_w, dtype=np.float32)
    o_w = np.asarray(o_w, dtype=np.float32)

    in_maps = []
    for c in range(NCORES):
        wqkv = np.ascontiguousarray(np.concatenate(
            [q_w[:, c * DQ:(c + 1) * DQ],
             k_w[:, c * HD:(c + 1) * HD],
             v_w[:, c * HD:(c + 1) * HD]], axis=1)).astype(NPBF16)
        owc = np.ascontiguousarray(
            o_w[c * DQ:(c + 1) * DQ, :].reshape(QH, 128, HIDDEN)
            .transpose(1, 0, 2)).astype(NPBF16)
        in_maps.append({"xT": xT, "wqkv": wqkv, "ow": owc,
                        "cos_t": cos_t, "sin_t": sin_t, "stair": stair})
    return in_maps


def assemble_output(outs_per_core):
    """outs_per_core[c] = [NTCH, 64, HIDDEN] bf16; stitch to [1, S, HIDDEN]."""
    rows = S // NTCH // NCORES
    full = np.empty((S, HIDDEN), dtype=np.float32)
    for c in range(NCORES):
        o = np.asarray(outs_per_core[c]).astype(np.float32)
        for g in range(NTCH):
            r0 = g * TCH + c * rows
            full[r0:r0 + rows] = o[g]
    return full.reshape(1, S, HIDDEN)


def run(inputs: dict, trace: bool = False):
    """Run on the 8 NeuronCores; returns (full_output, BassKernelResults)."""
    nc = build_program()
    in_maps = make_in_maps(**inputs)
    res = run_bass_kernel_spmd(nc, in_maps, core_ids=list(range(NCORES)),
                               trace=trace)
    full = assemble_output([res.results[c]["out"] for c in range(NCORES)])
    return full, res


def kernel(**inputs) -> np.ndarray:
    out, _ = run(inputs)
    return out


# revision 17
# speedup vs baseline: 1.1783x; 1.0621x over previous
# Mistral sliding-window attention (B=1, S=2048, H=4096, 32 q heads / 8 kv
# heads, window 4096 -> plain causal at this S) on 8 Trainium2 NeuronCores.
#
# Sharding: tensor-parallel over heads. Core c owns q heads 4c..4c+3 and kv
# head c; hidden_states replicated (host-transposed to [H, S] bf16).
#
# v2 design (dense-PE pipeline, bf16 data):
# - Phase A: QKV projection in 4 token chunks of 512, m-pairs accumulated in
#   three 2-bank PSUM tags; psum->SBUF staging copies on ACT free the banks
#   within ~1us so consecutive chunks never stall; RoPE runs on DVE from the
#   bf16 staging during the next chunk; V tiles are PE-transposed between
#   k-groups of the next chunk (off the critical path).
# - Phase B: per q-chunk c: attention for all 4 heads (scores in 3 rotating
#   2-bank psum tags so the PE never waits on the ACT exp; probabilities in
#   bf16; causal staircase mask multiplied on DVE; PV + ones-denominator
#   accumulate in one shared tag), then immediately the o_proj for token
#   block g=c: contributions of all 4 local heads accumulate IN PSUM
#   (no DVE adds), partial [512, 4096] written to DRAM in bf16, and a
#   ReduceScatter(add) over the 8 cores fires per block so collectives
#   overlap the next chunk's attention. Host stitches the 8x4x64-row
#   outputs.

from contextlib import ExitStack

import numpy as np
import ml_dtypes

import concourse.bacc as bacc
import concourse.bass as bass
import concourse.mybir as mybir
import concourse.tile as tile
from concourse.bass_utils import run_bass_kernel_spmd
from concourse.masks import make_identity

HIDDEN = 4096
NH = 32
NKV = 8
HD = 128
THETA = 10000.0
S = 2048
NCORES = 8

QH = NH // NCORES          # 4 q heads per core
DQ = QH * HD               # 512 (per-core q/attn width)
DOUT = DQ + 2 * HD         # 768 = q heads + k + v projection width
KT = HIDDEN // 128         # 32 contraction tiles
KG = 8                     # k-tiles per x DMA / inner k-group
TCH = 512                  # token chunk (matmul moving dim)
NTCH = S // TCH            # 4
KVT = S // 128             # 16 kv tiles
SCALE = 1.0 / float(np.sqrt(HD))

F32 = mybir.dt.float32
BF16 = mybir.dt.bfloat16
EXP = mybir.ActivationFunctionType.Exp
NPBF16 = ml_dtypes.bfloat16


def _rope(nc, rp, stg_half, qdst, cs2, sn2):
    """RoPE one [128, 512] head-tile: stg (bf16 SBUF) -> qdst (bf16 SBUF).

    qdst = stg*cs2 + rotate_half(stg)*sn2, with cs2 = [cos; cos] and
    sn2 = [-sin; sin] stacked on 128 partitions (host-precomputed), so all
    DVE ops are partition-aligned; the rotate is two SBUF->SBUF DMAs.
    """
    b = rp.tile([128, TCH], BF16, name="rope_b")
    nc.sync.dma_start(out=b[0:64, :], in_=stg_half[64:128, :])
    nc.sync.dma_start(out=b[64:128, :], in_=stg_half[0:64, :])
    ta = rp.tile([128, TCH], BF16, name="rope_t")
    tb = rp.tile([128, TCH], BF16, name="rope_u")
    nc.vector.tensor_mul(ta, stg_half, cs2)
    nc.vector.tensor_mul(tb, b, sn2)
    nc.vector.tensor_add(qdst, ta, tb)


def build_kernel_body(ctx: ExitStack, tc: tile.TileContext, outs, ins):
    nc = tc.nc
    xT, wqkv, ow, cos_t, sin_t, stair = (
        ins["xT"], ins["wqkv"], ins["ow"], ins["cos_t"], ins["sin_t"], ins["stair"],
    )
    out = outs["out"]

    # one partial tensor per token block so RS(g) never false-serializes
    # against the o_proj writes of block g+1
    partials = [nc.dram_tensor(f"partial{g}", [TCH, HIDDEN], BF16).ap()
                for g in range(NTCH)]
    rs_out = nc.dram_tensor("rs_out", [NTCH, S // NTCH // NCORES, HIDDEN],
                            BF16).ap()
    p3s = [p.rearrange("(b p) d -> p b d", p=128) for p in partials]

    singles = ctx.enter_context(tc.tile_pool(name="singles", bufs=1))
    # persistent bf16 state
    qT = singles.tile([128, QH, S], BF16)     # roped q, head h -> qT[:, h, :]
    kT = singles.tile([128, S], BF16)         # roped k
    V = singles.tile([128, KVT, HD], BF16)    # V[:, j, :] = [tok 128, d 128]
    ao = singles.tile([128, QH, S], BF16)     # attention out per head
    ones_sb = singles.tile([128, 128], BF16)
    ident_sb = singles.tile([128, 128], BF16)
    stair_sb = singles.tile([128, 896], BF16)
    cos_sb = singles.tile([128, S], BF16)
    sin_sb = singles.tile([128, S], BF16)
    ow_sb = singles.tile([128, QH, HIDDEN], BF16)   # o_w rows, d on partition

    # shared PSUM tags: three [128, 2, 512] f32 accumulators (6 banks)
    psh = ctx.enter_context(tc.tile_pool(name="psh", bufs=1, space="PSUM"))

    def T(i):
        return psh.tile([128, 2, TCH], F32, name=f"T{i}", tag=f"T{i}")

    # ---- DMA front matter ------------------------------------------------
    wq3 = wqkv.rearrange("(k p) d -> p k d", p=128)
    x3 = xT.rearrange("(k p) s -> p k s", p=128)

    with (
        tc.tile_pool(name="wq", bufs=1) as wp,
        tc.tile_pool(name="xt", bufs=4) as xp,
        tc.tile_pool(name="stg", bufs=6) as sp,
        tc.tile_pool(name="rope", bufs=3) as rp,
        tc.tile_pool(name="pva", bufs=1, space="PSUM") as ppv,
    ):
        # chunk-0 x tiles interleaved with the weight k-groups so the PE's
        # k-ordered consumption is never starved by a long weight queue
        w_sb = [wp.tile([128, DOUT], BF16, name=f"w{k}", tag=f"w{k}")
                for k in range(KT)]
        xg_c0 = []
        for kg in range(KT // KG):
            xg = xp.tile([128, KG, TCH], BF16, name="xg")
            if kg == 0:
                # split so the first k-tiles land within a few us
                nc.sync.dma_start(out=xg[:, 0:2, :], in_=x3[:, 0:2, 0:TCH])
                nc.sync.dma_start(out=xg[:, 2:KG, :], in_=x3[:, 2:KG, 0:TCH])
            else:
                nc.sync.dma_start(out=xg,
                                  in_=x3[:, kg * KG:(kg + 1) * KG, 0:TCH])
            xg_c0.append(xg)
            for k in range(kg * KG, (kg + 1) * KG):
                nc.sync.dma_start(out=w_sb[k], in_=wq3[:, k, :])
        nc.sync.dma_start(out=cos_sb, in_=cos_t)
        nc.sync.dma_start(out=sin_sb, in_=sin_t)
        nc.sync.dma_start(out=stair_sb, in_=stair)
        nc.vector.memset(ones_sb, 1.0)
        make_identity(nc, ident_sb)

        # ---- phase A: QKV projection + RoPE ------------------------------
        # stg tiles of previous chunk, consumed while the next chunk runs
        pend = None  # (chunk, [stg0, stg1, stg2])

        def drain_pending(after_kg):
            """Emit rope/transpose work for the previous chunk, interleaved
            into the current chunk's PE stream (after k-group `after_kg`)."""
            if pend is None:
                return
            c, stgs = pend
            lo = c * TCH
            cs = cos_sb[:, lo:lo + TCH]
            sn = sin_sb[:, lo:lo + TCH]
            if after_kg == 1:
                for mg in range(2):
                    for i in range(2):
                        h = 2 * mg + i
                        _rope(nc, rp, stgs[mg][:, i, :],
                              qT[:, h, lo:lo + TCH], cs, sn)
                _rope(nc, rp, stgs[2][:, 0, :], kT[:, lo:lo + TCH], cs, sn)
            elif after_kg in (2, 3):
                # two V transposes per k-group seam (PE, cheap, late-emitted)
                for t in range(2 * (after_kg - 2), 2 * (after_kg - 1)):
                    j = 4 * c + t
                    pv = ppv.tile([128, 128], BF16, name="pv", tag="pv")
                    nc.tensor.transpose(
                        pv, stgs[2][:, 1, t * 128:(t + 1) * 128], ident_sb)
                    nc.scalar.copy(out=V[:, j, :], in_=pv)

        for c in range(NTCH):
            lo = c * TCH
            pss = [T(mg) for mg in range(3)]
            for kg in range(KT // KG):
                if c == 0:
                    xg = xg_c0[kg]
                else:
                    xg = xp.tile([128, KG, TCH], BF16, name="xg")
                    nc.sync.dma_start(
                        out=xg,
                        in_=x3[:, kg * KG:(kg + 1) * KG, lo:lo + TCH])
                for mg in range(3):
                    for ki in range(KG):
                        k = kg * KG + ki
                        for i in range(2):
                            m = 2 * mg + i
                            nc.tensor.matmul(
                                pss[mg][:, i, :],
                                lhsT=w_sb[k][:, m * 128:(m + 1) * 128],
                                rhs=xg[:, ki, :],
                                start=(k == 0), stop=(k == KT - 1),
                            )
                drain_pending(kg)
            # copy psum -> bf16 staging (ACT), freeing banks for next chunk
            stgs = []
            for mg in range(3):
                stg = sp.tile([128, 2, TCH], BF16, name=f"stg{mg}")
                nc.scalar.copy(out=stg, in_=pss[mg])
                stgs.append(stg)
            pend = (c, stgs)
            if c == 0:
                # o_proj weights: 4.2MB, queued after chunk-0 supply so it
                # loads during chunks 1-2 without starving the projection
                nc.sync.dma_start(out=ow_sb, in_=ow)
        # drain the last chunk's rope + V transposes
        for kg in range(1, 4):
            drain_pending(kg)
        pend = None

    # ---- phase B: attention + o_proj + ReduceScatter ---------------------
    with (
        tc.tile_pool(name="pt", bufs=6) as ptp,
        tc.tile_pool(name="nrm", bufs=2) as nrmp,
        tc.tile_pool(name="ost", bufs=4) as ostp,
        tc.tile_pool(name="psb", bufs=1, space="PSUM") as psb,
    ):
        # score-psum tags rotate GLOBALLY across waves (not per head) so a
        # head's first scores reuse the oldest buffer - the one whose exp
        # finished ~2 waves ago - never the one still being consumed.
        wave_ctr = [0]

        def sc_tile(w):
            i = w % 3
            if i < 2:
                return T(i)
            return psb.tile([128, 2, TCH], F32, name="T3", tag="T3")

        def attention_chunk(h, c, pvq):
            jmax = 4 * c + 3
            npair = 2 * c + 2
            t2 = T(2)
            po = t2[:, 0, :]
            ps = t2[:, 1, :]
            qslice = qT[:, h, c * TCH:(c + 1) * TCH]

            def pv_wave(args):
                pt, p, po_, ps_ = args
                for i in range(2):
                    j = 2 * p + i
                    nc.tensor.matmul(po_, lhsT=V[:, j, :], rhs=pt[:, i, :],
                                     start=(j == 0), stop=(j == jmax))
                for i in range(2):
                    j = 2 * p + i
                    nc.tensor.matmul(ps_, lhsT=ones_sb, rhs=pt[:, i, :],
                                     start=(j == 0), stop=(j == jmax))

            for p in range(npair):
                sct = sc_tile(wave_ctr[0])
                wave_ctr[0] += 1
                for i in range(2):
                    j = 2 * p + i
                    nc.tensor.matmul(
                        sct[:, i, :], lhsT=kT[:, j * 128:(j + 1) * 128],
                        rhs=qslice, start=True, stop=True)
                pt = ptp.tile([128, 2, TCH], BF16, name="pt")
                nc.scalar.activation(pt, sct, EXP, scale=SCALE)
                for i in range(2):
                    j = 2 * p + i
                    rdiag = j - 4 * c
                    if rdiag >= 0:  # tile touches the causal diagonal
                        off = 384 - rdiag * 128
                        nc.vector.tensor_mul(
                            pt[:, i, :], pt[:, i, :],
                            stair_sb[:, off:off + TCH])
                pvq.append((pv_wave, (pt, p, po, ps), None))
                if len(pvq) > 2:
                    fn, args, fin = pvq.pop(0)
                    fn(args)
                    if fin is not None:
                        fin()
            # normalize once this head's last PV retires from the lag queue
            def finalize(h=h, c=c, po=po, ps=ps):
                rec = nrmp.tile([128, TCH], F32, name="rec")
                nc.vector.reciprocal(rec, ps)
                nc.vector.tensor_mul(ao[:, h, c * TCH:(c + 1) * TCH], po, rec)
            fn, args, fin = pvq[-1]
            pvq[-1] = (fn, args, finalize)

        def drain_pvq(pvq):
            while pvq:
                fn, args, fin = pvq.pop(0)
                fn(args)
                if fin is not None:
                    fin()

        def oproj_chunk(g):
            """partials[g] = local 4-head o_proj for tokens [g*512,(g+1)*512)."""
            for cc in range(HIDDEN // TCH):
                acc = [T(0), T(1)]
                for h in range(QH):
                    for t in range(4):
                        nc.tensor.matmul(
                            acc[t // 2][:, t % 2, :],
                            lhsT=ao[:, h, g * TCH + t * 128:
                                    g * TCH + (t + 1) * 128],
                            rhs=ow_sb[:, h, cc * TCH:(cc + 1) * TCH],
                            start=(h == 0), stop=(h == QH - 1),
                        )
                for half in range(2):
                    ost = ostp.tile([128, 2, TCH], BF16, name="ost")
                    if half == 0:
                        nc.scalar.copy(out=ost, in_=acc[0])
                    else:
                        nc.vector.tensor_copy(ost, acc[1])
                    nc.sync.dma_start(
                        out=p3s[g][:, 2 * half:2 * half + 2,
                                   cc * TCH:(cc + 1) * TCH],
                        in_=ost)

        for c in range(NTCH):
            pvq = []
            for h in range(QH):
                attention_chunk(h, c, pvq)
            drain_pvq(pvq)
            oproj_chunk(c)
            nc.gpsimd.collective_compute(
                "ReduceScatter",
                mybir.AluOpType.add,
                ins=[partials[c][:, :]],
                outs=[rs_out[c]],
                replica_groups=[list(range(NCORES))],
            )
        # final output copies go last: a DMA that waits on RS(c) parked on a
        # queue mid-stream would head-of-line-block later tile DMAs
        for c in range(NTCH):
            nc.sync.dma_start(out=out[c], in_=rs_out[c])


_NC_CACHE = None


def build_program():
    global _NC_CACHE
    if _NC_CACHE is not None:
        return _NC_CACHE
    nc = bacc.Bacc("TRN2", target_bir_lowering=False, debug=False,
                   num_devices=NCORES)
    ins = {
        "xT": nc.dram_tensor("xT", [HIDDEN, S], BF16, kind="ExternalInput").ap(),
        "wqkv": nc.dram_tensor("wqkv", [HIDDEN, DOUT], BF16,
                               kind="ExternalInput").ap(),
        "ow": nc.dram_tensor("ow", [128, QH, HIDDEN], BF16,
                             kind="ExternalInput").ap(),
        "cos_t": nc.dram_tensor("cos_t", [128, S], BF16, kind="ExternalInput").ap(),
        "sin_t": nc.dram_tensor("sin_t", [128, S], BF16, kind="ExternalInput").ap(),
        "stair": nc.dram_tensor("stair", [128, 896], BF16,
                                kind="ExternalInput").ap(),
    }
    outs = {"out": nc.dram_tensor(
        "out", [NTCH, S // NTCH // NCORES, HIDDEN], BF16,
        kind="ExternalOutput").ap()}
    with tile.TileContext(nc) as tc:
        with ExitStack() as ctx:
            build_kernel_body(ctx, tc, outs, ins)
    nc.compile()
    _NC_CACHE = nc
    return nc


def make_in_maps(hidden_states, position_ids, q_w, k_w, v_w, o_w):
    x = np.asarray(hidden_states, dtype=np.float32).reshape(S, HIDDEN)
    xT = np.ascontiguousarray(x.T).astype(NPBF16)
    pos = np.asarray(position_ids).reshape(S).astype(np.float64)
    inv = 1.0 / (THETA ** (np.arange(0, HD, 2, dtype=np.float64) / HD))
    fr = inv[:, None] * pos[None, :]                       # [64, S]
    cos_t = np.concatenate([np.cos(fr), np.cos(fr)], 0).astype(NPBF16)
    sin_t = np.concatenate([-np.sin(fr), np.sin(fr)], 0).astype(NPBF16)
    u = np.arange(896, dtype=np.int64)[None, :]
    kvi = np.arange(128, dtype=np.int64)[:, None]
    stair = ((u - kvi) >= 384).astype(NPBF16)              # [128, 896]

    q_w = np.asarray(q_w, dtype=np.float32)
    k_w = np.asarray(k_w, dtype=np.float32)
    v_w = np.asarray(v_w, dtype=np.float32)
    o_w = np.asarray(o_w, dtype=np.float32)

    in_maps = []
    for c in range(NCORES):
        wqkv = np.ascontiguousarray(np.concatenate(
            [q_w[:, c * DQ:(c + 1) * DQ],
             k_w[:, c * HD:(c + 1) * HD],
             v_w[:, c * HD:(c + 1) * HD]], axis=1)).astype(NPBF16)
        owc = np.ascontiguousarray(
            o_w[c * DQ:(c + 1) * DQ, :].reshape(QH, 128, HIDDEN)
            .transpose(1, 0, 2)).astype(NPBF16)
        in_maps.append({"xT": xT, "wqkv": wqkv, "ow": owc,
                        "cos_t": cos_t, "sin_t": sin_t, "stair": stair})
    return in_maps


def assemble_output(outs_per_core):
    """outs_per_core[c] = [NTCH, 64, HIDDEN] bf16; stitch to [1, S, HIDDEN]."""
    rows = S // NTCH // NCORES
    full = np.empty((S, HIDDEN), dtype=np.float32)
    for c in range(NCORES):
        o = np.asarray(outs_per_core[c]).astype(np.float32)
        for g in range(NTCH):
            r0 = g * TCH + c * rows
            full[r0:r0 + rows] = o[g]
    return full.reshape(1, S, HIDDEN)


def run(inputs: dict, trace: bool = False):
    """Run on the 8 NeuronCores; returns (full_output, BassKernelResults)."""
    nc = build_program()
    in_maps = make_in_maps(**inputs)
    res = run_bass_kernel_spmd(nc, in_maps, core_ids=list(range(NCORES)),
                               trace=trace)
    full = assemble_output([res.results[c]["out"] for c in range(NCORES)])
    return full, res


def kernel(**inputs) -> np.ndarray:
    out, _ = run(inputs)
    return out


# revision 18
# speedup vs baseline: 1.1840x; 1.0048x over previous
# Mistral sliding-window attention (B=1, S=2048, H=4096, 32 q heads / 8 kv
# heads, window 4096 -> plain causal at this S) on 8 Trainium2 NeuronCores.
#
# Sharding: tensor-parallel over heads. Core c owns q heads 4c..4c+3 and kv
# head c; hidden_states replicated (host-transposed to [H, S] bf16).
#
# v2 design (dense-PE pipeline, bf16 data):
# - Phase A: QKV projection in 4 token chunks of 512, m-pairs accumulated in
#   three 2-bank PSUM tags; psum->SBUF staging copies on ACT free the banks
#   within ~1us so consecutive chunks never stall; RoPE runs on DVE from the
#   bf16 staging during the next chunk; V tiles are PE-transposed between
#   k-groups of the next chunk (off the critical path).
# - Phase B: per q-chunk c: attention for all 4 heads (scores in 3 rotating
#   2-bank psum tags so the PE never waits on the ACT exp; probabilities in
#   bf16; causal staircase mask multiplied on DVE; PV + ones-denominator
#   accumulate in one shared tag), then immediately the o_proj for token
#   block g=c: contributions of all 4 local heads accumulate IN PSUM
#   (no DVE adds), partial [512, 4096] written to DRAM in bf16, and a
#   ReduceScatter(add) over the 8 cores fires per block so collectives
#   overlap the next chunk's attention. Host stitches the 8x4x64-row
#   outputs.

from contextlib import ExitStack

import numpy as np
import ml_dtypes

import concourse.bacc as bacc
import concourse.bass as bass
import concourse.mybir as mybir
import concourse.tile as tile
from concourse.bass_utils import run_bass_kernel_spmd
from concourse.masks import make_identity

HIDDEN = 4096
NH = 32
NKV = 8
HD = 128
THETA = 10000.0
S = 2048
NCORES = 8

QH = NH // NCORES          # 4 q heads per core
DQ = QH * HD               # 512 (per-core q/attn width)
DOUT = DQ + 2 * HD         # 768 = q heads + k + v projection width
KT = HIDDEN // 128         # 32 contraction tiles
KG = 8                     # k-tiles per x DMA / inner k-group
TCH = 512                  # token chunk (matmul moving dim)
NTCH = S // TCH            # 4
KVT = S // 128             # 16 kv tiles
SCALE = 1.0 / float(np.sqrt(HD))

F32 = mybir.dt.float32
BF16 = mybir.dt.bfloat16
EXP = mybir.ActivationFunctionType.Exp
NPBF16 = ml_dtypes.bfloat16


def _rope(nc, rp, stg_half, qdst, cs2, sn2):
    """RoPE one [128, 512] head-tile: stg (bf16 SBUF) -> qdst (bf16 SBUF).

    qdst = stg*cs2 + rotate_half(stg)*sn2, with cs2 = [cos; cos] and
    sn2 = [-sin; sin] stacked on 128 partitions (host-precomputed), so all
    DVE ops are partition-aligned; the rotate is two SBUF->SBUF DMAs.
    """
    b = rp.tile([128, TCH], BF16, name="rope_b")
    nc.sync.dma_start(out=b[0:64, :], in_=stg_half[64:128, :])
    nc.sync.dma_start(out=b[64:128, :], in_=stg_half[0:64, :])
    ta = rp.tile([128, TCH], BF16, name="rope_t")
    tb = rp.tile([128, TCH], BF16, name="rope_u")
    nc.vector.tensor_mul(ta, stg_half, cs2)
    nc.vector.tensor_mul(tb, b, sn2)
    nc.vector.tensor_add(qdst, ta, tb)


def build_kernel_body(ctx: ExitStack, tc: tile.TileContext, outs, ins):
    nc = tc.nc
    xT, wqkv, ow, cos_t, sin_t, stair = (
        ins["xT"], ins["wqkv"], ins["ow"], ins["cos_t"], ins["sin_t"], ins["stair"],
    )
    out = outs["out"]

    # one partial tensor per token block so RS(g) never false-serializes
    # against the o_proj writes of block g+1
    partials = [nc.dram_tensor(f"partial{g}", [TCH, HIDDEN], BF16).ap()
                for g in range(NTCH)]
    rs_out = nc.dram_tensor("rs_out", [NTCH, S // NTCH // NCORES, HIDDEN],
                            BF16).ap()
    p3s = [p.rearrange("(b p) d -> p b d", p=128) for p in partials]

    singles = ctx.enter_context(tc.tile_pool(name="singles", bufs=1))
    # persistent bf16 state
    qT = singles.tile([128, QH, S], BF16)     # roped q, head h -> qT[:, h, :]
    kT = singles.tile([128, S], BF16)         # roped k
    V = singles.tile([128, KVT, HD], BF16)    # V[:, j, :] = [tok 128, d 128]
    ao = singles.tile([128, QH, S], BF16)     # attention out per head
    ones_sb = singles.tile([128, 128], BF16)
    ident_sb = singles.tile([128, 128], BF16)
    stair_sb = singles.tile([128, 896], BF16)
    cos_sb = singles.tile([128, S], BF16)
    sin_sb = singles.tile([128, S], BF16)
    ow_sb = singles.tile([128, QH, HIDDEN], BF16)   # o_w rows, d on partition

    # shared PSUM tags: three [128, 2, 512] f32 accumulators (6 banks)
    psh = ctx.enter_context(tc.tile_pool(name="psh", bufs=1, space="PSUM"))

    def T(i):
        return psh.tile([128, 2, TCH], F32, name=f"T{i}", tag=f"T{i}")

    # ---- DMA front matter ------------------------------------------------
    wq3 = wqkv.rearrange("(k p) d -> p k d", p=128)
    x3 = xT.rearrange("(k p) s -> p k s", p=128)

    with (
        tc.tile_pool(name="wq", bufs=1) as wp,
        tc.tile_pool(name="xt", bufs=4) as xp,
        tc.tile_pool(name="stg", bufs=6) as sp,
        tc.tile_pool(name="rope", bufs=3) as rp,
        tc.tile_pool(name="pva", bufs=1, space="PSUM") as ppv,
    ):
        # chunk-0 x tiles interleaved with the weight k-groups so the PE's
        # k-ordered consumption is never starved by a long weight queue
        w_sb = [wp.tile([128, DOUT], BF16, name=f"w{k}", tag=f"w{k}")
                for k in range(KT)]
        xg_c0 = []
        for kg in range(KT // KG):
            xg = xp.tile([128, KG, TCH], BF16, name="xg")
            if kg == 0:
                # split so the first k-tiles land within a few us
                nc.sync.dma_start(out=xg[:, 0:2, :], in_=x3[:, 0:2, 0:TCH])
                nc.sync.dma_start(out=xg[:, 2:KG, :], in_=x3[:, 2:KG, 0:TCH])
            else:
                nc.sync.dma_start(out=xg,
                                  in_=x3[:, kg * KG:(kg + 1) * KG, 0:TCH])
            xg_c0.append(xg)
            for k in range(kg * KG, (kg + 1) * KG):
                nc.sync.dma_start(out=w_sb[k], in_=wq3[:, k, :])
        nc.sync.dma_start(out=cos_sb, in_=cos_t)
        nc.sync.dma_start(out=sin_sb, in_=sin_t)
        nc.sync.dma_start(out=stair_sb, in_=stair)
        nc.vector.memset(ones_sb, 1.0)
        make_identity(nc, ident_sb)

        # ---- phase A: QKV projection + RoPE ------------------------------
        # stg tiles of previous chunk, consumed while the next chunk runs
        pend = None  # (chunk, [stg0, stg1, stg2])

        def drain_pending(after_kg):
            """Emit rope/transpose work for the previous chunk, interleaved
            into the current chunk's PE stream (after k-group `after_kg`)."""
            if pend is None:
                return
            c, stgs = pend
            lo = c * TCH
            cs = cos_sb[:, lo:lo + TCH]
            sn = sin_sb[:, lo:lo + TCH]
            if after_kg == 1:
                for mg in range(2):
                    for i in range(2):
                        h = 2 * mg + i
                        _rope(nc, rp, stgs[mg][:, i, :],
                              qT[:, h, lo:lo + TCH], cs, sn)
                _rope(nc, rp, stgs[2][:, 0, :], kT[:, lo:lo + TCH], cs, sn)
            elif after_kg in (2, 3):
                # two V transposes per k-group seam (PE, cheap, late-emitted)
                for t in range(2 * (after_kg - 2), 2 * (after_kg - 1)):
                    j = 4 * c + t
                    pv = ppv.tile([128, 128], BF16, name="pv", tag="pv")
                    nc.tensor.transpose(
                        pv, stgs[2][:, 1, t * 128:(t + 1) * 128], ident_sb)
                    nc.scalar.copy(out=V[:, j, :], in_=pv)

        for c in range(NTCH):
            lo = c * TCH
            pss = [T(mg) for mg in range(3)]
            for kg in range(KT // KG):
                if c == 0:
                    xg = xg_c0[kg]
                else:
                    xg = xp.tile([128, KG, TCH], BF16, name="xg")
                    nc.sync.dma_start(
                        out=xg,
                        in_=x3[:, kg * KG:(kg + 1) * KG, lo:lo + TCH])
                for mg in range(3):
                    for ki in range(KG):
                        k = kg * KG + ki
                        for i in range(2):
                            m = 2 * mg + i
                            nc.tensor.matmul(
                                pss[mg][:, i, :],
                                lhsT=w_sb[k][:, m * 128:(m + 1) * 128],
                                rhs=xg[:, ki, :],
                                start=(k == 0), stop=(k == KT - 1),
                            )
                drain_pending(kg)
            # copy psum -> bf16 staging (ACT), freeing banks for next chunk
            stgs = []
            for mg in range(3):
                stg = sp.tile([128, 2, TCH], BF16, name=f"stg{mg}")
                nc.scalar.copy(out=stg, in_=pss[mg])
                stgs.append(stg)
            pend = (c, stgs)
            if c == 0:
                # o_proj weights: 4.2MB, queued after chunk-0 supply so it
                # loads during chunks 1-2 without starving the projection
                nc.sync.dma_start(out=ow_sb, in_=ow)
        # drain the last chunk's rope + V transposes
        for kg in range(1, 4):
            drain_pending(kg)
        pend = None

    # ---- phase B: attention + o_proj + ReduceScatter ---------------------
    with (
        tc.tile_pool(name="pt", bufs=8) as ptp,
        tc.tile_pool(name="nrm", bufs=2) as nrmp,
        tc.tile_pool(name="ost", bufs=4) as ostp,
        tc.tile_pool(name="psb", bufs=1, space="PSUM") as psb,
    ):
        # score-psum tags rotate GLOBALLY across waves (not per head) so a
        # head's first scores reuse the oldest buffer - the one whose exp
        # finished ~2 waves ago - never the one still being consumed.
        wave_ctr = [0]

        def sc_tile(w):
            i = w % 3
            if i < 2:
                return T(i)
            return psb.tile([128, 2, TCH], F32, name="T3", tag="T3")

        def attention_chunk(h, c, pvq):
            jmax = 4 * c + 3
            npair = 2 * c + 2
            t2 = T(2)
            po = t2[:, 0, :]
            ps = t2[:, 1, :]
            qslice = qT[:, h, c * TCH:(c + 1) * TCH]

            def pv_wave(args):
                pt, p, po_, ps_ = args
                for i in range(2):
                    j = 2 * p + i
                    nc.tensor.matmul(po_, lhsT=V[:, j, :], rhs=pt[:, i, :],
                                     start=(j == 0), stop=(j == jmax))
                nc.tensor.ldweights(ones_sb[:, :])
                for i in range(2):
                    j = 2 * p + i
                    nc.tensor.matmul(ps_, lhsT=ones_sb, rhs=pt[:, i, :],
                                     start=(j == 0), stop=(j == jmax))

            for p in range(npair):
                sct = sc_tile(wave_ctr[0])
                wave_ctr[0] += 1
                for i in range(2):
                    j = 2 * p + i
                    nc.tensor.matmul(
                        sct[:, i, :], lhsT=kT[:, j * 128:(j + 1) * 128],
                        rhs=qslice, start=True, stop=True)
                pt = ptp.tile([128, 2, TCH], BF16, name="pt")
                nc.scalar.activation(pt, sct, EXP, scale=SCALE)
                for i in range(2):
                    j = 2 * p + i
                    rdiag = j - 4 * c
                    if rdiag >= 0:  # tile touches the causal diagonal
                        off = 384 - rdiag * 128
                        nc.vector.tensor_mul(
                            pt[:, i, :], pt[:, i, :],
                            stair_sb[:, off:off + TCH])
                pvq.append((pv_wave, (pt, p, po, ps), None))
                if len(pvq) > 4:
                    fn, args, fin = pvq.pop(0)
                    fn(args)
                    if fin is not None:
                        fin()
            # normalize once this head's last PV retires from the lag queue
            def finalize(h=h, c=c, po=po, ps=ps):
                rec = nrmp.tile([128, TCH], F32, name="rec")
                nc.vector.reciprocal(rec, ps)
                nc.vector.tensor_mul(ao[:, h, c * TCH:(c + 1) * TCH], po, rec)
            fn, args, fin = pvq[-1]
            pvq[-1] = (fn, args, finalize)

        def drain_pvq(pvq):
            while pvq:
                fn, args, fin = pvq.pop(0)
                fn(args)
                if fin is not None:
                    fin()

        def oproj_chunk(g):
            """partials[g] = local 4-head o_proj for tokens [g*512,(g+1)*512)."""
            for cc in range(HIDDEN // TCH):
                acc = [T(0), T(1)]
                for h in range(QH):
                    for t in range(4):
                        nc.tensor.matmul(
                            acc[t // 2][:, t % 2, :],
                            lhsT=ao[:, h, g * TCH + t * 128:
                                    g * TCH + (t + 1) * 128],
                            rhs=ow_sb[:, h, cc * TCH:(cc + 1) * TCH],
                            start=(h == 0), stop=(h == QH - 1),
                        )
                for half in range(2):
                    ost = ostp.tile([128, 2, TCH], BF16, name="ost")
                    if half == 0:
                        nc.scalar.copy(out=ost, in_=acc[0])
                    else:
                        nc.vector.tensor_copy(ost, acc[1])
                    nc.sync.dma_start(
                        out=p3s[g][:, 2 * half:2 * half + 2,
                                   cc * TCH:(cc + 1) * TCH],
                        in_=ost)

        for c in range(NTCH):
            pvq = []
            for h in range(QH):
                attention_chunk(h, c, pvq)
            drain_pvq(pvq)
            oproj_chunk(c)
            nc.gpsimd.collective_compute(
                "ReduceScatter",
                mybir.AluOpType.add,
                ins=[partials[c][:, :]],
                outs=[rs_out[c]],
                replica_groups=[list(range(NCORES))],
            )
            # on the gpsimd queue (behind RS(c)) so it cannot head-of-line
            # block the sync-queue tile DMAs
            nc.gpsimd.dma_start(out=out[c], in_=rs_out[c])


_NC_CACHE = None


def build_program():
    global _NC_CACHE
    if _NC_CACHE is not None:
        return _NC_CACHE
    nc = bacc.Bacc("TRN2", target_bir_lowering=False, debug=False,
                   num_devices=NCORES)
    ins = {
        "xT": nc.dram_tensor("xT", [HIDDEN, S], BF16, kind="ExternalInput").ap(),
        "wqkv": nc.dram_tensor("wqkv", [HIDDEN, DOUT], BF16,
                               kind="ExternalInput").ap(),
        "ow": nc.dram_tensor("ow", [128, QH, HIDDEN], BF16,
                             kind="ExternalInput").ap(),
        "cos_t": nc.dram_tensor("cos_t", [128, S], BF16, kind="ExternalInput").ap(),
        "sin_t": nc.dram_tensor("sin_t", [128, S], BF16, kind="ExternalInput").ap(),
        "stair": nc.dram_tensor("stair", [128, 896], BF16,
                                kind="ExternalInput").ap(),
    }
    outs = {"out": nc.dram_tensor(
        "out", [NTCH, S // NTCH // NCORES, HIDDEN], BF16,
        kind="ExternalOutput").ap()}
    with tile.TileContext(nc) as tc:
        with ExitStack() as ctx:
            build_kernel_body(ctx, tc, outs, ins)
    nc.compile()
    _NC_CACHE = nc
    return nc


def make_in_maps(hidden_states, position_ids, q_w, k_w, v_w, o_w):
    x = np.asarray(hidden_states, dtype=np.float32).reshape(S, HIDDEN)
    xT = np.ascontiguousarray(x.T).astype(NPBF16)
    pos = np.asarray(position_ids).reshape(S).astype(np.float64)
    inv = 1.0 / (THETA ** (np.arange(0, HD, 2, dtype=np.float64) / HD))
    fr = inv[:, None] * pos[None, :]                       # [64, S]
    cos_t = np.concatenate([np.cos(fr), np.cos(fr)], 0).astype(NPBF16)
    sin_t = np.concatenate([-np.sin(fr), np.sin(fr)], 0).astype(NPBF16)
    u = np.arange(896, dtype=np.int64)[None, :]
    kvi = np.arange(128, dtype=np.int64)[:, None]
    stair = ((u - kvi) >= 384).astype(NPBF16)              # [128, 896]

    q_w = np.asarray(q_w, dtype=np.float32)
    k_w = np.asarray(k_w, dtype=np.float32)
    v_w = np.asarray(v_w, dtype=np.float32)
    o_w = np.asarray(o_w, dtype=np.float32)

    in_maps = []
    for c in range(NCORES):
        wqkv = np.ascontiguousarray(np.concatenate(
            [q_w[:, c * DQ:(c + 1) * DQ],
             k_w[:, c * HD:(c + 1) * HD],
             v_w[:, c * HD:(c + 1) * HD]], axis=1)).astype(NPBF16)
        owc = np.ascontiguousarray(
            o_w[c * DQ:(c + 1) * DQ, :].reshape(QH, 128, HIDDEN)
            .transpose(1, 0, 2)).astype(NPBF16)
        in_maps.append({"xT": xT, "wqkv": wqkv, "ow": owc,
                        "cos_t": cos_t, "sin_t": sin_t, "stair": stair})
    return in_maps


def assemble_output(outs_per_core):
    """outs_per_core[c] = [NTCH, 64, HIDDEN] bf16; stitch to [1, S, HIDDEN]."""
    rows = S // NTCH // NCORES
    full = np.empty((S, HIDDEN), dtype=np.float32)
    for c in range(NCORES):
        o = np.asarray(outs_per_core[c]).astype(np.float32)
        for g in range(NTCH):
            r0 = g * TCH + c * rows
            full[r0:r0 + rows] = o[g]
    return full.reshape(1, S, HIDDEN)


def run(inputs: dict, trace: bool = False):
    """Run on the 8 NeuronCores; returns (full_output, BassKernelResults)."""
    nc = build_program()
    in_maps = make_in_maps(**inputs)
    res = run_bass_kernel_spmd(nc, in_maps, core_ids=list(range(NCORES)),
                               trace=trace)
    full = assemble_output([res.results[c]["out"] for c in range(NCORES)])
    return full, res


def kernel(**inputs) -> np.ndarray:
    out, _ = run(inputs)
    return out
